# revision 30
# baseline (speedup 1.0000x reference)
import sys

sys.path.insert(0, "/opt/trn_rl_repo")

import numpy as np

# Problem constants (nn_Attention_34978213658826)
B, L, DM, NH, DH = 2, 2048, 1024, 16, 64
P = 128
LT = L // P            # 16 q/k tiles
MC = DM // P           # 8 m-chunks
G = 4                  # q-tiles per group for the z matmul
NG = LT // G
HPC = 4                # heads per core
NPAIR = 2              # head pairs per core
NEG = -1.0e30
SCH = 1024             # scores psum chunk width

_CACHE = {}


def _ts(i, n):
    return slice(i * n, (i + 1) * n)


def build_bass():
    import concourse.mybir as mybir
    import concourse.tile as tile
    from concourse import bacc

    f32 = mybir.dt.float32
    bf16 = mybir.dt.bfloat16
    AX = mybir.AxisListType
    AF = mybir.ActivationFunctionType

    nc = bacc.Bacc(None, target_bir_lowering=False)
    # x^T split hi/lo in bf16 (hi + lo ~= fp32-accurate contraction)
    xh_d = nc.dram_tensor("xh", [DM, L], bf16, kind="ExternalInput")
    xl_d = nc.dram_tensor("xl", [DM, L], bf16, kind="ExternalInput")
    wq_h = nc.dram_tensor("wqh", [NPAIR, DM + 1, P], bf16, kind="ExternalInput")
    wq_l = nc.dram_tensor("wql", [NPAIR, DM + 1, P], bf16, kind="ExternalInput")
    wk_h = nc.dram_tensor("wkh", [NPAIR, DM + 1, P], bf16, kind="ExternalInput")
    wk_l = nc.dram_tensor("wkl", [NPAIR, DM + 1, P], bf16, kind="ExternalInput")
    wv_d = nc.dram_tensor("wv", [DM + 1, HPC * DH], bf16, kind="ExternalInput")
    wo_d = nc.dram_tensor("wo", [NPAIR, P, DM], bf16, kind="ExternalInput")
    msk = nc.dram_tensor("mask", [P, P], bf16, kind="ExternalInput")
    idn = nc.dram_tensor("ident", [P, P], bf16, kind="ExternalInput")
    out = nc.dram_tensor("out", [L, DM], bf16, kind="ExternalOutput")
    wu_d = nc.dram_tensor("wu", [1, 1], f32, kind="ExternalOutput")

    with tile.TileContext(nc) as tc:
        with (
            tc.tile_pool(name="const", bufs=1) as const,
            tc.tile_pool(name="w", bufs=1) as wp,
            tc.tile_pool(name="qk", bufs=1) as qkp,
            tc.tile_pool(name="vz", bufs=1) as vzp,
        ):
            ident = const.tile([P, P], bf16)
            nc.sync.dma_start(ident, idn[:, :])
            mask = const.tile([P, P], bf16)
            nc.sync.dma_start(mask, msk[:, :])
            ones = const.tile([2, 512], bf16)
            nc.vector.memset(ones, 1.0)

            # weights: [partition=m-row, pair, m-chunk, headcol]
            wqk = {}
            for nm in ("qh", "ql", "kh", "kl"):
                t = wp.tile([P, NPAIR, MC, P], bf16, name=f"w{nm}", tag=f"w{nm}")
                wqk[nm] = t
            # bias rows stacked hi/lo on 2 partitions
            bq2 = wp.tile([2, NPAIR, P], bf16)
            bk2 = wp.tile([2, NPAIR, P], bf16)
            wv_t = wp.tile([P, MC, HPC * DH], bf16)
            wv_b = wp.tile([1, HPC * DH], bf16)
            wo_t = wp.tile([P, NPAIR, DM], bf16)

            # per-head stacked layouts: rows 0:64 hi, 64:128 lo (kSw swapped)
            qS = qkp.tile([P, HPC, L], bf16)
            kS = qkp.tile([P, HPC, L], bf16)
            kSw = qkp.tile([P, HPC, L], bf16)
            vv = vzp.tile([P, LT, HPC * DH], bf16)
            zst = [vzp.tile([P, NPAIR, G * P], bf16, name=f"zst{g}", tag=f"zst{g}") for g in range(NG)]

            # xh outlives xl: v-projection runs interleaved with early score
            # blocks, so xh stays allocated; xl frees before the score pools.
            xh = vzp.tile([P, MC, L], bf16)
            xlp_ctx = tc.tile_pool(name="xlt", bufs=1)
            xlp = xlp_ctx.__enter__()
            xl = xlp.tile([P, MC, L], bf16)

            # pair-layout projection outputs (transient, freed before scores)
            prp_ctx = tc.tile_pool(name="prt", bufs=1)
            prp = prp_ctx.__enter__()
            qTh = prp.tile([P, NPAIR, L], bf16)
            qTl = prp.tile([P, NPAIR, L], bf16)
            kTh = prp.tile([P, NPAIR, L], bf16)
            kTl = prp.tile([P, NPAIR, L], bf16)

            # ---------------- DMA emission (sync queue paces x chunks) -----
            nc.sync.dma_start(xh[:, 0], xh_d[_ts(0, P), :])
            nc.sync.dma_start(xl[:, 0], xl_d[_ts(0, P), :])
            for nm, dram in (("qh", wq_h), ("ql", wq_l), ("kh", wk_h), ("kl", wk_l)):
                nc.sync.dma_start(
                    wqk[nm][:, 0], dram[0, :DM, :].rearrange("(c p) h -> p c h", p=P)
                )
            nc.sync.dma_start(bq2[0:1, 0], wq_h[0, DM : DM + 1, :])
            nc.sync.dma_start(bq2[1:2, 0], wq_l[0, DM : DM + 1, :])
            nc.sync.dma_start(bk2[0:1, 0], wk_h[0, DM : DM + 1, :])
            nc.sync.dma_start(bk2[1:2, 0], wk_l[0, DM : DM + 1, :])
            for m in range(1, MC):
                nc.sync.dma_start(xh[:, m], xh_d[_ts(m, P), :])
                nc.sync.dma_start(xl[:, m], xl_d[_ts(m, P), :])
            for nm, dram in (("qh", wq_h), ("ql", wq_l), ("kh", wk_h), ("kl", wk_l)):
                nc.sync.dma_start(
                    wqk[nm][:, 1], dram[1, :DM, :].rearrange("(c p) h -> p c h", p=P)
                )
            nc.sync.dma_start(bq2[0:1, 1], wq_h[1, DM : DM + 1, :])
            nc.sync.dma_start(bq2[1:2, 1], wq_l[1, DM : DM + 1, :])
            nc.sync.dma_start(bk2[0:1, 1], wk_h[1, DM : DM + 1, :])
            nc.sync.dma_start(bk2[1:2, 1], wk_l[1, DM : DM + 1, :])
            nc.sync.dma_start(wv_t, wv_d[:DM, :].rearrange("(c p) h -> p c h", p=P))
            nc.sync.dma_start(wv_b, wv_d[DM : DM + 1, :])
            for _pr in range(NPAIR):
                nc.sync.dma_start(wo_t[:, _pr], wo_d[_pr, :, :])

            with (
                tc.tile_pool(name="proj_ps", bufs=8, space="PSUM") as proj_ps,
            ):
                # PE warm-up: dummy matmuls while inputs stream in (p-state).
                wup_st = tc.tile_pool(name="wupp", bufs=1)
                wupp = wup_st.__enter__()
                wup = wupp.tile([1, 4], f32)
                wps = proj_ps.tile([P, 512], f32, name="wps", tag="pp")
                for w_ in range(48):
                    nc.tensor.matmul(
                        wps[:, :P], lhsT=ident, rhs=mask,
                        start=(w_ == 0), stop=(w_ == 47),
                    )
                nc.vector.reduce_max(wup[:1, :1], wps[:1, :P], axis=AX.X)
                nc.sync.dma_start(wu_d[:, :], wup[:1, :1])
                wup_st.__exit__(None, None, None)

                NQ = L // 512

                def proj_pair_phase(pr):
                    # q-proj and k-proj for pair pr, m-major, 8 psum banks
                    pps = {}
                    for w_, _b, _dh, _dl in (("q", bq2, qTh, qTl), ("k", bk2, kTh, kTl)):
                        for n in range(NQ):
                            pps[(w_, n)] = proj_ps.tile([P, 512], f32, name="pp", tag="pp")
                    for m in range(MC):
                        for w_ in ("q", "k"):
                            th = wqk[w_ + "h"]
                            tl = wqk[w_ + "l"]
                            for vi, (lw, rx) in enumerate((
                                (th[:, pr, m, :], xh),
                                (tl[:, pr, m, :], xh),
                                (th[:, pr, m, :], xl),
                            )):
                                for n in range(NQ):
                                    nc.tensor.matmul(
                                        pps[(w_, n)], lhsT=lw,
                                        rhs=rx[:, m, _ts(n, 512)],
                                        start=(m == 0 and vi == 0), stop=False,
                                    )
                    for w_, b2, dh, dl in (("q", bq2, qTh, qTl), ("k", bk2, kTh, kTl)):
                        for n in range(NQ):
                            ps = pps[(w_, n)]
                            nc.tensor.matmul(
                                ps, lhsT=b2[:, pr, :], rhs=ones[:, :512],
                                start=False, stop=True,
                            )
                            nc.scalar.copy(dh[:, pr, _ts(n, 512)], ps)
                            nc.vector.scalar_tensor_tensor(
                                dl[:, pr, _ts(n, 512)], ps, 1.0,
                                dh[:, pr, _ts(n, 512)],
                                op0=mybir.AluOpType.mult,
                                op1=mybir.AluOpType.subtract,
                            )
                    # build per-head stacked tiles via sbuf-sbuf DMA
                    for h2 in range(2):
                        h = pr * 2 + h2
                        hs = _ts(h2, DH)
                        nc.sync.dma_start(qS[0:DH, h, :], qTh[hs, pr, :])
                        nc.sync.dma_start(qS[DH:P, h, :], qTl[hs, pr, :])
                        nc.sync.dma_start(kS[0:DH, h, :], kTh[hs, pr, :])
                        nc.sync.dma_start(kS[DH:P, h, :], kTl[hs, pr, :])
                        nc.sync.dma_start(kSw[0:DH, h, :], kTl[hs, pr, :])
                        nc.sync.dma_start(kSw[DH:P, h, :], kTh[hs, pr, :])

                proj_pair_phase(0)
                proj_pair_phase(1)

            prp_ctx.__exit__(None, None, None)
            xlp_ctx.__exit__(None, None, None)

            # ---------------- fused score/z/out stages ----------------
            with (
                tc.tile_pool(name="s_ps", bufs=3, space="PSUM") as s_ps,
                tc.tile_pool(name="zo_ps", bufs=2, space="PSUM") as zo_ps,
                tc.tile_pool(name="prow", bufs=3) as prowp,
                tc.tile_pool(name="pt", bufs=2) as ptp,
                tc.tile_pool(name="stat", bufs=8) as statp,
                tc.tile_pool(name="osb", bufs=2) as osbp,
            ):
                from collections import deque

                ptgs = {}
                vq = deque(range(LT))

                def pump_v(n):
                    # v-projection tiles as PE filler in the early, latency-
                    # bound score blocks (uses the z/O psum pool).
                    for _ in range(n):
                        if not vq:
                            return
                        lt = vq.popleft()
                        ps = zo_ps.tile([P, 512], f32, name="vps", tag="zo")
                        for m in range(MC):
                            nc.tensor.matmul(
                                ps[:, : HPC * DH],
                                lhsT=xh[:, m, _ts(lt, P)], rhs=wv_t[:, m, :],
                                start=(m == 0), stop=False,
                            )
                        nc.tensor.matmul(
                            ps[:, : HPC * DH],
                            lhsT=ones[0:1, :P], rhs=wv_b,
                            start=False, stop=True,
                        )
                        nc.scalar.copy(vv[:, lt, :], ps[:, : HPC * DH])

                def emit_S_front(pr, g, s, h2):
                    # one (head, q-tile): score matmuls + per-chunk max + exp
                    if s == 0 and h2 == 0:
                        ptgs[(pr, g)] = [
                            ptp.tile([P, LT, G, P], bf16, name=f"ptg{j}", tag=f"ptg{j}")
                            for j in range(2)
                        ]
                    ptg = ptgs[(pr, g)]
                    h = pr * 2 + h2
                    i = g * G + s
                    klen = (i + 1) * P
                    nch = (klen + SCH - 1) // SCH
                    lq = qS[:, h, _ts(i, P)]
                    prow = prowp.tile([P, L], bf16)
                    negmc = statp.tile([P, 4], f32, tag="negmc")
                    sums = statp.tile([P, 4], f32, tag="sums")
                    for c in range(nch):
                        cw = min(SCH, klen - c * SCH)
                        dlo = klen - P - c * SCH  # diag block offset in chunk
                        sp = s_ps.tile([P, SCH], f32, name="sp", tag="s")
                        for w0 in range(0, cw, 512):
                            ww = min(512, cw - w0)
                            has_diag = w0 <= dlo < w0 + ww
                            nc.tensor.matmul(
                                sp[:, w0 : w0 + ww], lhsT=lq,
                                rhs=kS[:, h, c * SCH + w0 : c * SCH + w0 + ww],
                                start=True, stop=False,
                            )
                            nc.tensor.matmul(
                                sp[:, w0 : w0 + ww], lhsT=lq,
                                rhs=kSw[:, h, c * SCH + w0 : c * SCH + w0 + ww],
                                start=False, stop=not has_diag,
                            )
                            if has_diag:
                                nc.tensor.matmul(
                                    sp[:, dlo : dlo + P], lhsT=ident, rhs=mask,
                                    start=False, stop=True,
                                )
                        nc.vector.reduce_max(
                            negmc[:, c : c + 1], sp[:, :cw], axis=AX.X, negate=True
                        )
                        nc.scalar.activation(
                            prow[:, c * SCH : c * SCH + cw],
                            sp[:, :cw],
                            AF.Exp,
                            bias=negmc[:, c : c + 1],
                            accum_out=sums[:, c : c + 1],
                        )
                    return (pr, g, s, h2, prow, negmc, sums, nch, klen)

                def emit_S_back(ctx):
                    # deferred per-unit tail: global rescale + 1/sum + transpose
                    pr, g, s, h2, prow, negmc, sums, nch, klen = ctx
                    i = g * G + s
                    ptg = ptgs[(pr, g)]
                    sinv = statp.tile([P, 1], f32, tag="sinv")
                    if nch > 1:
                        negmg = statp.tile([P, 1], f32, tag="negmg")
                        nc.vector.tensor_reduce(
                            negmg, negmc[:, :nch], axis=AX.X, op=mybir.AluOpType.min
                        )
                        rsc = statp.tile([P, 4], f32, tag="rsc")
                        nc.scalar.activation(
                            rsc[:, :nch], negmc[:, :nch], AF.Exp,
                            bias=negmg, scale=-1.0,
                        )
                        ssc = statp.tile([P, 4], f32, tag="ssc")
                        nc.vector.tensor_mul(ssc[:, :nch], sums[:, :nch], rsc[:, :nch])
                        stot = statp.tile([P, 1], f32, tag="stot")
                        nc.vector.reduce_sum(stot, ssc[:, :nch], axis=AX.X)
                        nc.vector.reciprocal(sinv, stot)
                        wsc = statp.tile([P, 4], f32, tag="wsc")
                        nc.vector.tensor_scalar_mul(wsc[:, :nch], rsc[:, :nch], sinv)
                        for c in range(nch):
                            cw = min(SCH, klen - c * SCH)
                            nc.vector.tensor_scalar_mul(
                                prow[:, c * SCH : c * SCH + cw],
                                prow[:, c * SCH : c * SCH + cw],
                                wsc[:, c : c + 1],
                            )
                    else:
                        nc.vector.reciprocal(sinv, sums[:, :1])
                        nc.vector.tensor_scalar_mul(
                            prow[:, :klen], prow[:, :klen], sinv
                        )
                    nc.sync.dma_start_transpose(
                        ptg[h2][:, : i + 1, s, :], prow[:, :klen]
                    )

                def emit_Z_h(pr, g, h2):
                    ptg = ptgs[(pr, g)]
                    hcol = (pr * 2 + h2) * DH
                    zps = zo_ps.tile([DH, G * P], f32, name="zps", tag="zo")
                    jmax = G * (g + 1)
                    for j in range(jmax):
                        sc = max(0, j - G * g)
                        nc.tensor.matmul(
                            zps[:, sc * P :],
                            lhsT=vv[:, j, hcol : hcol + DH],
                            rhs=ptg[h2][:, j, sc:G, :],
                            start=(j == 0),
                            stop=(j == jmax - 1),
                        )
                    nc.scalar.copy(zst[g][_ts(h2, DH), pr, :], zps)

                def emit_O_qtile(g, s):
                    i = g * G + s
                    osb = osbp.tile([P, DM], bf16)
                    for mc2 in range(2):
                        ops = zo_ps.tile([P, 512], f32, name="ops", tag="zo")
                        for pr in range(NPAIR):
                            nc.tensor.matmul(
                                ops,
                                lhsT=zst[g][:, pr, _ts(s, P)],
                                rhs=wo_t[:, pr, _ts(mc2, 512)],
                                start=(pr == 0),
                                stop=(pr == 1),
                            )
                        nc.scalar.copy(osb[:, _ts(mc2, 512)], ops)
                    nc.gpsimd.dma_start(out[_ts(i, P), :], osb)

                # Deferred z / out-proj work popped between score units so the
                # PE always has independent matmuls while softmax drains.
                filler = deque()
                oq = deque()
                epoch = [0]

                def pump(n, drain=False):
                    for _ in range(n):
                        if filler and (drain or filler[0][0] <= epoch[0] - 1):
                            pr_, g_, h2_ = filler.popleft()[1]
                            emit_Z_h(pr_, g_, h2_)
                            if h2_ == 1 and pr_ == 1:
                                for s_ in range(G):
                                    oq.append((g_, s_))
                        elif len(oq) > 8:
                            emit_O_qtile(*oq.popleft())
                        else:
                            return

                pending = [None]

                def emit_S(pr, g):
                    # software pipeline: unit u+1's matmuls+max+exp are emitted
                    # before unit u's stat/rescale/transpose tail, so the DVE
                    # queue serves the next max (freeing PSUM via exp) before
                    # the previous unit's stat chain.
                    for s_ in range(G):
                        for h2 in range(2):
                            ctx = emit_S_front(pr, g, s_, h2)
                            if pending[0] is not None:
                                emit_S_back(pending[0])
                            pending[0] = ctx
                            pump_v(2)
                            pump(2)
                    for h2 in range(2):
                        filler.append((epoch[0], (pr, g, h2)))
                    epoch[0] += 1

                for g in range(NG):
                    for pr in range(NPAIR):
                        emit_S(pr, g)
                if pending[0] is not None:
                    emit_S_back(pending[0])
                    pending[0] = None
                # tail drain: deferred O-projections are independent PE work
                # that covers the last z chains' softmax/transpose latency.
                for _ in range(4):
                    if oq:
                        emit_O_qtile(*oq.popleft())
                while filler:
                    pump(1, drain=True)
                    for _ in range(4):
                        if oq:
                            emit_O_qtile(*oq.popleft())
                while oq:
                    emit_O_qtile(*oq.popleft())

    nc.finalize()
    return nc


def _split_bf16(a):
    import ml_dtypes

    hi = a.astype(ml_dtypes.bfloat16)
    lo = (a - hi.astype(np.float32)).astype(ml_dtypes.bfloat16)
    return hi, lo


def make_in_maps(normal_pre_resid, W_Q, W_K, W_V, W_O, b_Q, b_K, b_V, b_O):
    import ml_dtypes

    x = np.asarray(normal_pre_resid, np.float32)
    W_Q = np.asarray(W_Q, np.float32) * 0.125  # fold 1/sqrt(d_head)
    W_K = np.asarray(W_K, np.float32)
    W_V = np.asarray(W_V, np.float32)
    W_O = np.asarray(W_O, np.float32)
    b_Q = np.asarray(b_Q, np.float32) * 0.125
    b_K = np.asarray(b_K, np.float32)
    b_V = np.asarray(b_V, np.float32)

    mask = np.triu(np.full((P, P), NEG, np.float32), k=1).astype(ml_dtypes.bfloat16)
    ident = np.eye(P, dtype=np.float32).astype(ml_dtypes.bfloat16)
    in_maps = []
    for c in range(8):
        b, hg = divmod(c, 4)
        heads = [4 * hg + j for j in range(HPC)]
        xT = np.ascontiguousarray(x[b].T)  # [DM, L]
        xh, xl = _split_bf16(xT)

        def pack_qk(W, bias):
            prs = []
            for p_ in range(NPAIR):
                h0, h1 = heads[2 * p_], heads[2 * p_ + 1]
                wcat = np.concatenate([W[h0], W[h1]], axis=1)  # [DM, 128]
                bcat = np.concatenate([bias[h0], bias[h1]])[None, :]
                prs.append(np.concatenate([wcat, bcat], axis=0))  # [DM+1, 128]
            return _split_bf16(np.ascontiguousarray(np.stack(prs)))

        wqh, wql = pack_qk(W_Q, b_Q)
        wkh, wkl = pack_qk(W_K, b_K)
        wv_cat = np.concatenate([W_V[h] for h in heads], axis=1)
        bv_cat = np.concatenate([b_V[h] for h in heads])[None, :]
        wv_full = np.concatenate([wv_cat, bv_cat], axis=0).astype(ml_dtypes.bfloat16)
        wo_prs = np.ascontiguousarray(
            np.stack(
                [
                    np.concatenate(
                        [W_O[heads[2 * p_]], W_O[heads[2 * p_ + 1]]], axis=0
                    )
                    for p_ in range(NPAIR)
                ]
            )
        ).astype(ml_dtypes.bfloat16)  # [2, 128, DM]

        in_maps.append(
            {
                "xh": np.ascontiguousarray(xh),
                "xl": np.ascontiguousarray(xl),
                "wqh": wqh,
                "wql": wql,
                "wkh": wkh,
                "wkl": wkl,
                "wv": np.ascontiguousarray(wv_full),
                "wo": wo_prs,
                "mask": mask,
                "ident": ident,
            }
        )
    return in_maps


def run_device(in_maps, **kwargs):
    from concourse.bass_utils import run_bass_kernel_spmd

    if "nc" not in _CACHE:
        _CACHE["nc"] = build_bass()
    return run_bass_kernel_spmd(_CACHE["nc"], in_maps, core_ids=list(range(8)), **kwargs)


def kernel(normal_pre_resid, W_Q, W_K, W_V, W_O, b_Q, b_K, b_V, b_O, **extra):
    b_O = np.asarray(b_O, np.float32)
    in_maps = make_in_maps(
        normal_pre_resid, W_Q, W_K, W_V, W_O, b_Q, b_K, b_V, b_O
    )
    res = run_device(in_maps)
    outs = [r["out"] for r in res.results]
    full = np.zeros((B, L, DM), np.float32)
    for c in range(8):
        full[c // 4] += outs[c].astype(np.float32)
    full += b_O[None, None, :]
    return full


# revision 31
# speedup vs baseline: 1.0886x; 1.0886x over previous
import sys

sys.path.insert(0, "/opt/trn_rl_repo")

import numpy as np

# Problem constants (nn_Attention_34978213658826)
B, L, DM, NH, DH = 2, 2048, 1024, 16, 64
P = 128
LT = L // P            # 16 q/k tiles
MC = DM // P           # 8 m-chunks
G = 4                  # q-tiles per group for the z matmul
NG = LT // G
HPC = 4                # heads per core
NPAIR = 2              # head pairs per core
NEG = -1.0e30
SCH = 1024             # scores psum chunk width

_CACHE = {}


def _ts(i, n):
    return slice(i * n, (i + 1) * n)


def build_bass():
    import concourse.mybir as mybir
    import concourse.tile as tile
    from concourse import bacc

    f32 = mybir.dt.float32
    bf16 = mybir.dt.bfloat16
    AX = mybir.AxisListType
    AF = mybir.ActivationFunctionType

    nc = bacc.Bacc(None, target_bir_lowering=False)
    # x^T split hi/lo in bf16 (hi + lo ~= fp32-accurate contraction)
    xh_d = nc.dram_tensor("xh", [DM, L], bf16, kind="ExternalInput")
    xl_d = nc.dram_tensor("xl", [DM, L], bf16, kind="ExternalInput")
    wq_h = nc.dram_tensor("wqh", [NPAIR, DM + 1, P], bf16, kind="ExternalInput")
    wq_l = nc.dram_tensor("wql", [NPAIR, DM + 1, P], bf16, kind="ExternalInput")
    wk_h = nc.dram_tensor("wkh", [NPAIR, DM + 1, P], bf16, kind="ExternalInput")
    wk_l = nc.dram_tensor("wkl", [NPAIR, DM + 1, P], bf16, kind="ExternalInput")
    wv_d = nc.dram_tensor("wv", [DM + 1, HPC * DH], bf16, kind="ExternalInput")
    wo_d = nc.dram_tensor("wo", [NPAIR, P, DM], bf16, kind="ExternalInput")
    msk = nc.dram_tensor("mask", [P, P], bf16, kind="ExternalInput")
    idn = nc.dram_tensor("ident", [P, P], bf16, kind="ExternalInput")
    out = nc.dram_tensor("out", [L, DM], bf16, kind="ExternalOutput")
    wu_d = nc.dram_tensor("wu", [1, 1], f32, kind="ExternalOutput")

    with tile.TileContext(nc) as tc:
        with (
            tc.tile_pool(name="const", bufs=1) as const,
            tc.tile_pool(name="w", bufs=1) as wp,
            tc.tile_pool(name="qk", bufs=1) as qkp,
            tc.tile_pool(name="vz", bufs=1) as vzp,
        ):
            ident = const.tile([P, P], bf16)
            nc.sync.dma_start(ident, idn[:, :])
            mask = const.tile([P, P], bf16)
            nc.sync.dma_start(mask, msk[:, :])
            ones = const.tile([2, 512], bf16)
            nc.vector.memset(ones, 1.0)

            # weights: [partition=m-row, pair, m-chunk, headcol]
            wqk = {}
            for nm in ("qh", "ql", "kh", "kl"):
                t = wp.tile([P, NPAIR, MC, P], bf16, name=f"w{nm}", tag=f"w{nm}")
                wqk[nm] = t
            # bias rows stacked hi/lo on 2 partitions
            bq2 = wp.tile([2, NPAIR, P], bf16)
            bk2 = wp.tile([2, NPAIR, P], bf16)
            wv_t = wp.tile([P, MC, HPC * DH], bf16)
            wv_b = wp.tile([1, HPC * DH], bf16)
            wo_t = wp.tile([P, NPAIR, DM], bf16)

            # per-head stacked layouts: rows 0:64 hi, 64:128 lo (kSw swapped)
            qS = qkp.tile([P, HPC, L], bf16)
            kS = qkp.tile([P, HPC, L], bf16)
            kSw = qkp.tile([P, HPC, L], bf16)
            vv = vzp.tile([P, LT, HPC * DH], bf16)
            zst = [vzp.tile([P, NPAIR, G * P], bf16, name=f"zst{g}", tag=f"zst{g}") for g in range(NG)]

            xtp_ctx = tc.tile_pool(name="xt", bufs=1)
            xtp = xtp_ctx.__enter__()
            xh = xtp.tile([P, MC, L], bf16)
            xl = xtp.tile([P, MC, L], bf16)

            # pair-layout projection outputs (transient, freed before scores)
            prp_ctx = tc.tile_pool(name="prt", bufs=1)
            prp = prp_ctx.__enter__()
            qTh = prp.tile([P, NPAIR, L], bf16)
            qTl = prp.tile([P, NPAIR, L], bf16)
            kTh = prp.tile([P, NPAIR, L], bf16)
            kTl = prp.tile([P, NPAIR, L], bf16)

            # ---------------- DMA emission (sync queue paces x chunks) -----
            nc.sync.dma_start(xh[:, 0], xh_d[_ts(0, P), :])
            nc.sync.dma_start(xl[:, 0], xl_d[_ts(0, P), :])
            for nm, dram in (("qh", wq_h), ("ql", wq_l), ("kh", wk_h), ("kl", wk_l)):
                nc.sync.dma_start(
                    wqk[nm][:, 0], dram[0, :DM, :].rearrange("(c p) h -> p c h", p=P)
                )
            nc.sync.dma_start(bq2[0:1, 0], wq_h[0, DM : DM + 1, :])
            nc.sync.dma_start(bq2[1:2, 0], wq_l[0, DM : DM + 1, :])
            nc.sync.dma_start(bk2[0:1, 0], wk_h[0, DM : DM + 1, :])
            nc.sync.dma_start(bk2[1:2, 0], wk_l[0, DM : DM + 1, :])
            for m in range(1, MC):
                nc.sync.dma_start(xh[:, m], xh_d[_ts(m, P), :])
                nc.sync.dma_start(xl[:, m], xl_d[_ts(m, P), :])
            for nm, dram in (("qh", wq_h), ("ql", wq_l), ("kh", wk_h), ("kl", wk_l)):
                nc.sync.dma_start(
                    wqk[nm][:, 1], dram[1, :DM, :].rearrange("(c p) h -> p c h", p=P)
                )
            nc.sync.dma_start(bq2[0:1, 1], wq_h[1, DM : DM + 1, :])
            nc.sync.dma_start(bq2[1:2, 1], wq_l[1, DM : DM + 1, :])
            nc.sync.dma_start(bk2[0:1, 1], wk_h[1, DM : DM + 1, :])
            nc.sync.dma_start(bk2[1:2, 1], wk_l[1, DM : DM + 1, :])
            nc.sync.dma_start(wv_t, wv_d[:DM, :].rearrange("(c p) h -> p c h", p=P))
            nc.sync.dma_start(wv_b, wv_d[DM : DM + 1, :])
            for _pr in range(NPAIR):
                nc.sync.dma_start(wo_t[:, _pr], wo_d[_pr, :, :])

            with (
                tc.tile_pool(name="proj_ps", bufs=8, space="PSUM") as proj_ps,
            ):
                # PE warm-up: dummy matmuls while inputs stream in (p-state).
                wup_st = tc.tile_pool(name="wupp", bufs=1)
                wupp = wup_st.__enter__()
                wup = wupp.tile([1, 4], f32)
                wps = proj_ps.tile([P, 512], f32, name="wps", tag="pp")
                for w_ in range(48):
                    nc.tensor.matmul(
                        wps[:, :P], lhsT=ident, rhs=mask,
                        start=(w_ == 0), stop=(w_ == 47),
                    )
                nc.vector.reduce_max(wup[:1, :1], wps[:1, :P], axis=AX.X)
                nc.sync.dma_start(wu_d[:, :], wup[:1, :1])
                wup_st.__exit__(None, None, None)

                NQ = L // 512

                def proj_pair_phase(pr):
                    # q-proj and k-proj for pair pr, m-major, 8 psum banks
                    pps = {}
                    for w_, _b, _dh, _dl in (("q", bq2, qTh, qTl), ("k", bk2, kTh, kTl)):
                        for n in range(NQ):
                            pps[(w_, n)] = proj_ps.tile([P, 512], f32, name="pp", tag="pp")
                    for m in range(MC):
                        for w_ in ("q", "k"):
                            th = wqk[w_ + "h"]
                            tl = wqk[w_ + "l"]
                            for vi, (lw, rx) in enumerate((
                                (th[:, pr, m, :], xh),
                                (tl[:, pr, m, :], xh),
                                (th[:, pr, m, :], xl),
                            )):
                                for n in range(NQ):
                                    nc.tensor.matmul(
                                        pps[(w_, n)], lhsT=lw,
                                        rhs=rx[:, m, _ts(n, 512)],
                                        start=(m == 0 and vi == 0), stop=False,
                                    )
                    for w_, b2, dh, dl in (("q", bq2, qTh, qTl), ("k", bk2, kTh, kTl)):
                        for n in range(NQ):
                            ps = pps[(w_, n)]
                            nc.tensor.matmul(
                                ps, lhsT=b2[:, pr, :], rhs=ones[:, :512],
                                start=False, stop=True,
                            )
                            nc.scalar.copy(dh[:, pr, _ts(n, 512)], ps)
                            nc.vector.scalar_tensor_tensor(
                                dl[:, pr, _ts(n, 512)], ps, 1.0,
                                dh[:, pr, _ts(n, 512)],
                                op0=mybir.AluOpType.mult,
                                op1=mybir.AluOpType.subtract,
                            )
                    # build per-head stacked tiles via sbuf-sbuf DMA
                    for h2 in range(2):
                        h = pr * 2 + h2
                        hs = _ts(h2, DH)
                        nc.sync.dma_start(qS[0:DH, h, :], qTh[hs, pr, :])
                        nc.sync.dma_start(qS[DH:P, h, :], qTl[hs, pr, :])
                        nc.sync.dma_start(kS[0:DH, h, :], kTh[hs, pr, :])
                        nc.sync.dma_start(kS[DH:P, h, :], kTl[hs, pr, :])
                        nc.sync.dma_start(kSw[0:DH, h, :], kTl[hs, pr, :])
                        nc.sync.dma_start(kSw[DH:P, h, :], kTh[hs, pr, :])

                proj_pair_phase(0)
                proj_pair_phase(1)

                def v_proj():
                    for lt in range(LT):
                        ps = proj_ps.tile([P, 512], f32, name="vps", tag="pp")
                        for m in range(MC):
                            nc.tensor.matmul(
                                ps[:, : HPC * DH],
                                lhsT=xh[:, m, _ts(lt, P)], rhs=wv_t[:, m, :],
                                start=(m == 0), stop=False,
                            )
                        nc.tensor.matmul(
                            ps[:, : HPC * DH],
                            lhsT=ones[0:1, :P], rhs=wv_b,
                            start=False, stop=True,
                        )
                        nc.scalar.copy(vv[:, lt, :], ps[:, : HPC * DH])

                v_proj()

            prp_ctx.__exit__(None, None, None)
            xtp_ctx.__exit__(None, None, None)

            # ---------------- fused score/z/out stages ----------------
            with (
                tc.tile_pool(name="s_ps", bufs=3, space="PSUM") as s_ps,
                tc.tile_pool(name="zo_ps", bufs=2, space="PSUM") as zo_ps,
                tc.tile_pool(name="prow", bufs=5) as prowp,
                tc.tile_pool(name="pt", bufs=2) as ptp,
                tc.tile_pool(name="stat", bufs=8) as statp,
                tc.tile_pool(name="osb", bufs=2) as osbp,
            ):
                from collections import deque

                ptgs = {}

                def emit_S_front(pr, g, s, h2):
                    # one (head, q-tile): score matmuls + per-chunk max + exp
                    if s == 0 and h2 == 0:
                        ptgs[(pr, g)] = [
                            ptp.tile([P, LT, G, P], bf16, name=f"ptg{j}", tag=f"ptg{j}")
                            for j in range(2)
                        ]
                    ptg = ptgs[(pr, g)]
                    h = pr * 2 + h2
                    i = g * G + s
                    klen = (i + 1) * P
                    nch = (klen + SCH - 1) // SCH
                    lq = qS[:, h, _ts(i, P)]
                    prow = prowp.tile([P, L], bf16)
                    negmc = statp.tile([P, 4], f32, tag="negmc")
                    sums = statp.tile([P, 4], f32, tag="sums")
                    for c in range(nch):
                        cw = min(SCH, klen - c * SCH)
                        dlo = klen - P - c * SCH  # diag block offset in chunk
                        sp = s_ps.tile([P, SCH], f32, name="sp", tag="s")
                        for w0 in range(0, cw, 512):
                            ww = min(512, cw - w0)
                            has_diag = w0 <= dlo < w0 + ww
                            nc.tensor.matmul(
                                sp[:, w0 : w0 + ww], lhsT=lq,
                                rhs=kS[:, h, c * SCH + w0 : c * SCH + w0 + ww],
                                start=True, stop=False,
                            )
                            nc.tensor.matmul(
                                sp[:, w0 : w0 + ww], lhsT=lq,
                                rhs=kSw[:, h, c * SCH + w0 : c * SCH + w0 + ww],
                                start=False, stop=not has_diag,
                            )
                            if has_diag:
                                nc.tensor.matmul(
                                    sp[:, dlo : dlo + P], lhsT=ident, rhs=mask,
                                    start=False, stop=True,
                                )
                        nc.vector.reduce_max(
                            negmc[:, c : c + 1], sp[:, :cw], axis=AX.X, negate=True
                        )
                        nc.scalar.activation(
                            prow[:, c * SCH : c * SCH + cw],
                            sp[:, :cw],
                            AF.Exp,
                            bias=negmc[:, c : c + 1],
                            accum_out=sums[:, c : c + 1],
                        )
                    return (pr, g, s, h2, prow, negmc, sums, nch, klen)

                def emit_S_back(ctx):
                    # deferred per-unit tail: global rescale + 1/sum + transpose
                    pr, g, s, h2, prow, negmc, sums, nch, klen = ctx
                    i = g * G + s
                    ptg = ptgs[(pr, g)]
                    sinv = statp.tile([P, 1], f32, tag="sinv")
                    if nch > 1:
                        negmg = statp.tile([P, 1], f32, tag="negmg")
                        nc.vector.tensor_reduce(
                            negmg, negmc[:, :nch], axis=AX.X, op=mybir.AluOpType.min
                        )
                        rsc = statp.tile([P, 4], f32, tag="rsc")
                        nc.scalar.activation(
                            rsc[:, :nch], negmc[:, :nch], AF.Exp,
                            bias=negmg, scale=-1.0,
                        )
                        ssc = statp.tile([P, 4], f32, tag="ssc")
                        nc.vector.tensor_mul(ssc[:, :nch], sums[:, :nch], rsc[:, :nch])
                        stot = statp.tile([P, 1], f32, tag="stot")
                        nc.vector.reduce_sum(stot, ssc[:, :nch], axis=AX.X)
                        nc.vector.reciprocal(sinv, stot)
                        wsc = statp.tile([P, 4], f32, tag="wsc")
                        nc.vector.tensor_scalar_mul(wsc[:, :nch], rsc[:, :nch], sinv)
                        for c in range(nch):
                            cw = min(SCH, klen - c * SCH)
                            nc.vector.tensor_scalar_mul(
                                prow[:, c * SCH : c * SCH + cw],
                                prow[:, c * SCH : c * SCH + cw],
                                wsc[:, c : c + 1],
                            )
                    else:
                        nc.vector.reciprocal(sinv, sums[:, :1])
                        nc.vector.tensor_scalar_mul(
                            prow[:, :klen], prow[:, :klen], sinv
                        )
                    nc.sync.dma_start_transpose(
                        ptg[h2][:, : i + 1, s, :], prow[:, :klen]
                    )

                def emit_Z_h(pr, g, h2):
                    ptg = ptgs[(pr, g)]
                    hcol = (pr * 2 + h2) * DH
                    zps = zo_ps.tile([DH, G * P], f32, name="zps", tag="zo")
                    jmax = G * (g + 1)
                    for j in range(jmax):
                        sc = max(0, j - G * g)
                        nc.tensor.matmul(
                            zps[:, sc * P :],
                            lhsT=vv[:, j, hcol : hcol + DH],
                            rhs=ptg[h2][:, j, sc:G, :],
                            start=(j == 0),
                            stop=(j == jmax - 1),
                        )
                    nc.scalar.copy(zst[g][_ts(h2, DH), pr, :], zps)

                def emit_O_qtile(g, s):
                    i = g * G + s
                    osb = osbp.tile([P, DM], bf16)
                    for mc2 in range(2):
                        ops = zo_ps.tile([P, 512], f32, name="ops", tag="zo")
                        for pr in range(NPAIR):
                            nc.tensor.matmul(
                                ops,
                                lhsT=zst[g][:, pr, _ts(s, P)],
                                rhs=wo_t[:, pr, _ts(mc2, 512)],
                                start=(pr == 0),
                                stop=(pr == 1),
                            )
                        nc.scalar.copy(osb[:, _ts(mc2, 512)], ops)
                    nc.gpsimd.dma_start(out[_ts(i, P), :], osb)

                # Deferred z / out-proj work popped between score units so the
                # PE always has independent matmuls while softmax drains.
                filler = deque()
                oq = deque()
                epoch = [0]

                def pump(n, drain=False):
                    for _ in range(n):
                        if filler and (drain or filler[0][0] <= epoch[0] - 1):
                            pr_, g_, h2_ = filler.popleft()[1]
                            emit_Z_h(pr_, g_, h2_)
                            if h2_ == 1 and pr_ == 1:
                                for s_ in range(G):
                                    oq.append((g_, s_))
                        elif len(oq) > 8:
                            emit_O_qtile(*oq.popleft())
                        else:
                            return

                pending = [None]

                def emit_S(pr, g):
                    # software pipeline: unit u+1's matmuls+max+exp are emitted
                    # before unit u's stat/rescale/transpose tail, so the DVE
                    # queue serves the next max (freeing PSUM via exp) before
                    # the previous unit's stat chain.
                    for s_ in range(G):
                        for h2 in range(2):
                            ctx = emit_S_front(pr, g, s_, h2)
                            if pending[0] is not None:
                                emit_S_back(pending[0])
                            pending[0] = ctx
                            pump(2)
                    for h2 in range(2):
                        filler.append((epoch[0], (pr, g, h2)))
                    epoch[0] += 1

                for g in range(NG):
                    for pr in range(NPAIR):
                        emit_S(pr, g)
                if pending[0] is not None:
                    emit_S_back(pending[0])
                    pending[0] = None
                # tail drain: deferred O-projections are independent PE work
                # that covers the last z chains' softmax/transpose latency.
                for _ in range(4):
                    if oq:
                        emit_O_qtile(*oq.popleft())
                while filler:
                    pump(1, drain=True)
                    for _ in range(4):
                        if oq:
                            emit_O_qtile(*oq.popleft())
                while oq:
                    emit_O_qtile(*oq.popleft())

    nc.finalize()
    return nc


def _split_bf16(a):
    import ml_dtypes

    hi = a.astype(ml_dtypes.bfloat16)
    lo = (a - hi.astype(np.float32)).astype(ml_dtypes.bfloat16)
    return hi, lo


def make_in_maps(normal_pre_resid, W_Q, W_K, W_V, W_O, b_Q, b_K, b_V, b_O):
    import ml_dtypes

    x = np.asarray(normal_pre_resid, np.float32)
    W_Q = np.asarray(W_Q, np.float32) * 0.125  # fold 1/sqrt(d_head)
    W_K = np.asarray(W_K, np.float32)
    W_V = np.asarray(W_V, np.float32)
    W_O = np.asarray(W_O, np.float32)
    b_Q = np.asarray(b_Q, np.float32) * 0.125
    b_K = np.asarray(b_K, np.float32)
    b_V = np.asarray(b_V, np.float32)

    mask = np.triu(np.full((P, P), NEG, np.float32), k=1).astype(ml_dtypes.bfloat16)
    ident = np.eye(P, dtype=np.float32).astype(ml_dtypes.bfloat16)
    in_maps = []
    for c in range(8):
        b, hg = divmod(c, 4)
        heads = [4 * hg + j for j in range(HPC)]
        xT = np.ascontiguousarray(x[b].T)  # [DM, L]
        xh, xl = _split_bf16(xT)

        def pack_qk(W, bias):
            prs = []
            for p_ in range(NPAIR):
                h0, h1 = heads[2 * p_], heads[2 * p_ + 1]
                wcat = np.concatenate([W[h0], W[h1]], axis=1)  # [DM, 128]
                bcat = np.concatenate([bias[h0], bias[h1]])[None, :]
                prs.append(np.concatenate([wcat, bcat], axis=0))  # [DM+1, 128]
            return _split_bf16(np.ascontiguousarray(np.stack(prs)))

        wqh, wql = pack_qk(W_Q, b_Q)
        wkh, wkl = pack_qk(W_K, b_K)
        wv_cat = np.concatenate([W_V[h] for h in heads], axis=1)
        bv_cat = np.concatenate([b_V[h] for h in heads])[None, :]
        wv_full = np.concatenate([wv_cat, bv_cat], axis=0).astype(ml_dtypes.bfloat16)
        wo_prs = np.ascontiguousarray(
            np.stack(
                [
                    np.concatenate(
                        [W_O[heads[2 * p_]], W_O[heads[2 * p_ + 1]]], axis=0
                    )
                    for p_ in range(NPAIR)
                ]
            )
        ).astype(ml_dtypes.bfloat16)  # [2, 128, DM]

        in_maps.append(
            {
                "xh": np.ascontiguousarray(xh),
                "xl": np.ascontiguousarray(xl),
                "wqh": wqh,
                "wql": wql,
                "wkh": wkh,
                "wkl": wkl,
                "wv": np.ascontiguousarray(wv_full),
                "wo": wo_prs,
                "mask": mask,
                "ident": ident,
            }
        )
    return in_maps


def run_device(in_maps, **kwargs):
    from concourse.bass_utils import run_bass_kernel_spmd

    if "nc" not in _CACHE:
        _CACHE["nc"] = build_bass()
    return run_bass_kernel_spmd(_CACHE["nc"], in_maps, core_ids=list(range(8)), **kwargs)


def kernel(normal_pre_resid, W_Q, W_K, W_V, W_O, b_Q, b_K, b_V, b_O, **extra):
    b_O = np.asarray(b_O, np.float32)
    in_maps = make_in_maps(
        normal_pre_resid, W_Q, W_K, W_V, W_O, b_Q, b_K, b_V, b_O
    )
    res = run_device(in_maps)
    outs = [r["out"] for r in res.results]
    full = np.zeros((B, L, DM), np.float32)
    for c in range(8):
        full[c // 4] += outs[c].astype(np.float32)
    full += b_O[None, None, :]
    return full
